# revision 21
# baseline (speedup 1.0000x reference)
"""Trainium2 Bass kernel for an equivariant attention block (GNN message passing).

Computes, for N=1024 nodes with hidden H=128:
    rel_x     = x[:,None,:] - x[None,:,:]                      [N,N,2]
    edge_feat = relu(rel_x @ We1 + be1) @ We2 + be2            [N,N,H]
    q,k,v     = h@Wq+bq, h@Wk+bk, h@Wv+bv
    attn      = softmax(q k^T / sqrt(H))
    agg_h     = attn @ v
    gate      = edge_feat @ Wc + bc                            [N,N,1]
    delta_x   = sum_j attn[:,:,None]*gate*rel_x                [N,2]
    returns (h + agg_h, x + delta_x)

Key algebraic restructuring (exact):
    u = x @ We1;  w2c = We2 @ Wc;  c0 = be2 @ Wc + bc
    gate[i,j] = sum_h w2c[h]*relu(u[i,h]-u[j,h]+be1[h]) + c0
    delta_x[i] = ( (sum_j s[i,j]) * x[i] - (s @ x)[i] ) / Z[i]
        where s = E * (gate + c0), E = exp(scores), Z = rowsum(E)
so the [N,N,H] edge tensor is never materialized; the only O(N^2 H) work is
the per-pair relu + weighted H-reduction, done fully on-chip.

Per query row i a relu tile [h=128, j=N] is built on ScalarE/VectorE and
reduced over h by an M=32 accumulating f32r matmul whose stationary operand
is w2c placed in column (i mod 32) — 32 rows of gate land consolidated in one
[32, N] PSUM tile per row-group.

Sharding: rows (queries) split across 8 NeuronCores, 128 rows each; k/v and
params replicated; no collectives.
"""

import numpy as np

import concourse.bacc as bacc
import concourse.bass as bass
import concourse.mybir as mybir
import concourse.tile as tile
from concourse.bass_utils import run_bass_kernel_spmd

F32 = mybir.dt.float32
F32R = mybir.dt.float32r

N = 1024
H = 128
NCORES = 8
R = N // NCORES  # rows per core = 128

# fraction of relu rows on the scalar (ACT) engine; rest on DVE
ACT_ROWS = set(i for i in range(R) if (i % 8) < 3)


def build_nc():
    nc = bacc.Bacc()

    # ---- per-core DRAM parameters (inputs) ----
    dp = lambda name, shape: nc.declare_dram_parameter(name, shape, F32, isOutput=False)
    hT_d = dp("hT", [H, N])          # h^T, replicated
    hTo_d = dp("hT_own", [H, R])     # own columns of h^T
    ho_d = dp("h_own", [R, H])       # own rows of h
    xo_d = dp("x_own", [R, 2])
    xaug_d = dp("x_aug", [128, 8 * 3])   # [p, b, (x0,x1,1)]
    xT_d = dp("xT", [2, N])
    xTo_d = dp("xT_own", [2, R])
    wq_d = dp("Wq_s", [H, H])        # pre-scaled by 1/sqrt(H)
    wk_d = dp("Wk", [H, H])
    wv_d = dp("Wv", [H, H])
    we1_d = dp("We1", [2, H])
    # f32r end-to-end: DMA'd bits are fp32; the PE reads them in fast-fp32 mode
    w2cd_d = nc.declare_dram_parameter("W2CD", [H, 32 * 32], F32R, isOutput=False)
    id_d = dp("ident", [128, 128])
    bqc_d = dp("bq_col", [H, 1])
    bkc_d = dp("bk_col", [H, 1])
    be1c_d = dp("be1_col", [H, 1])
    c0c_d = dp("c0_col", [128, 1])
    bvr_d = dp("bv_row", [1, H])
    ones_d = dp("ones_col", [1, 128])

    out_h_d = nc.declare_dram_parameter("out_h", [R, H], F32, isOutput=True)
    out_x_d = nc.declare_dram_parameter("out_x", [R, 2], F32, isOutput=True)

    with tile.TileContext(nc) as tc:
        with (
            tc.tile_pool(name="const", bufs=1) as cpool,
            tc.tile_pool(name="work", bufs=1) as wpool,
            tc.tile_pool(name="relu", bufs=6) as rpool,
        ):
            # ---- load constants/inputs to SBUF ----
            def load(pool, dram, shape, name):
                t = pool.tile(shape, F32, tag=name)
                nc.sync.dma_start(out=t[:], in_=dram[:])
                return t

            hT = load(cpool, hT_d, [H, N], "hT")
            hTo = load(cpool, hTo_d, [H, R], "hTo")
            ho = load(cpool, ho_d, [R, H], "ho")
            xo = load(cpool, xo_d, [R, 2], "xo")
            xaug = load(cpool, xaug_d, [128, 24], "xaug")
            xT = load(cpool, xT_d, [2, N], "xT")
            xTo = load(cpool, xTo_d, [2, R], "xTo")
            wq = load(cpool, wq_d, [H, H], "wq")
            wk = load(cpool, wk_d, [H, H], "wk")
            wv = load(cpool, wv_d, [H, H], "wv")
            we1 = load(cpool, we1_d, [2, H], "we1")
            w2cd = cpool.tile([H, 1024], F32R, tag="w2cd")
            nc.sync.dma_start(out=w2cd[:], in_=w2cd_d[:])
            ident = load(cpool, id_d, [128, 128], "ident")
            bqc = load(cpool, bqc_d, [H, 1], "bqc")
            bkc = load(cpool, bkc_d, [H, 1], "bkc")
            be1c = load(cpool, be1c_d, [H, 1], "be1c")
            c0c = load(cpool, c0c_d, [128, 1], "c0c")
            bvr = load(cpool, bvr_d, [1, H], "bvr")
            onesc = load(cpool, ones_d, [1, 128], "onesc")

            E = wpool.tile([R, N], F32, tag="E")
            Z = wpool.tile([R, 1], F32, tag="Z")
            rZ = wpool.tile([R, 1], F32, tag="rZ")
            uT = wpool.tile([H, N], F32, tag="uT")
            uTbo = wpool.tile([H, R], F32, tag="uTbo")
            nuT = wpool.tile([H, N], F32, tag="nuT")
            qT = wpool.tile([H, R], F32R, tag="qT")
            kT = wpool.tile([H, N], F32R, tag="kT")
            vnat = wpool.tile([128, N], F32, tag="vnat")
            ET = wpool.tile([128, N], F32, tag="ET")
            s_sb = wpool.tile([R, N], F32, tag="s")
            oh = wpool.tile([R, H], F32, tag="oh")

            with (
                tc.tile_pool(name="ppbig", bufs=2, space="PSUM") as ppbig,
                tc.tile_pool(name="pptp", bufs=2, space="PSUM") as pptp,
                tc.tile_pool(name="ppagg", bufs=1, space="PSUM") as ppagg,
            ):
                # ---- phase A: u, q, k, v ----
                # uT[h, n] = sum_c We1[c, h] * xT[c, n]
                pu = ppbig.tile([128, N], F32, tag="big")
                for f in range(2):
                    nc.tensor.matmul(
                        out=pu[:, f * 512:(f + 1) * 512],
                        lhsT=we1[:], rhs=xT[:, f * 512:(f + 1) * 512],
                    )
                nc.vector.tensor_copy(uT[:], pu[:])
                # uTb_own[h, i] = u[global_i, h] + be1[h]  (bias cols, own rows)
                puo = pptp.tile([128, 128], F32, tag="tp")
                nc.tensor.matmul(out=puo[:, 0:R], lhsT=we1[:], rhs=xTo[:])
                nc.vector.tensor_scalar(
                    out=uTbo[:], in0=puo[:, 0:R], scalar1=be1c[:], scalar2=None,
                    op0=mybir.AluOpType.add,
                )
                nc.vector.tensor_scalar(
                    out=nuT[:], in0=uT[:], scalar1=-1.0, scalar2=None,
                    op0=mybir.AluOpType.mult,
                )

                # qT[h_out, i] (own rows only), bias added on copy-out
                pq = pptp.tile([128, 128], F32, tag="tp")
                nc.tensor.matmul(out=pq[:], lhsT=wq[:], rhs=hTo[:])
                nc.scalar.activation(
                    qT[:], pq[:], mybir.ActivationFunctionType.Identity,
                    bias=bqc[:],
                )

                # kT[h_out, n] full
                pk = ppbig.tile([128, N], F32, tag="big")
                for f in range(2):
                    nc.tensor.matmul(
                        out=pk[:, f * 512:(f + 1) * 512],
                        lhsT=wk[:], rhs=hT[:, f * 512:(f + 1) * 512],
                    )
                for f in range(2):
                    nc.scalar.activation(
                        kT[:, f * 512:(f + 1) * 512], pk[:, f * 512:(f + 1) * 512],
                        mybir.ActivationFunctionType.Identity, bias=bkc[:],
                    )

                # v in natural layout: block b holds v[b*128:(b+1)*128, :]
                pv = ppbig.tile([128, N], F32, tag="big")
                for b in range(8):
                    sl = slice(b * 128, (b + 1) * 128)
                    nc.tensor.matmul(out=pv[:, sl], lhsT=hT[:, sl], rhs=wv[:],
                                     start=True, stop=False)
                    nc.tensor.matmul(out=pv[:, sl], lhsT=onesc[:], rhs=bvr[:],
                                     start=False, stop=True)
                nc.vector.tensor_copy(vnat[:], pv[:])

                # ---- phase B: attention ----
                ps = ppbig.tile([128, N], F32, tag="big")
                for f in range(2):
                    nc.tensor.matmul(
                        out=ps[:, f * 512:(f + 1) * 512],
                        lhsT=qT[:], rhs=kT[:, f * 512:(f + 1) * 512],
                    )
                nc.scalar.activation(
                    E[:], ps[:], mybir.ActivationFunctionType.Exp, accum_out=Z[:],
                )
                nc.vector.reciprocal(rZ[:], Z[:])

                # E^T blocks (for agg matmul lhsT)
                for b in range(8):
                    sl = slice(b * 128, (b + 1) * 128)
                    pt = pptp.tile([128, 128], F32, tag="tp")
                    nc.tensor.transpose(pt[:], E[:, sl], ident[:])
                    nc.vector.tensor_copy(ET[:, sl], pt[:])

                # agg[i, h] = sum_j E[i,j] v[j,h], accumulated over blocks
                pagg = ppagg.tile([R, H], F32, tag="agg")
                for b in range(8):
                    sl = slice(b * 128, (b + 1) * 128)
                    nc.tensor.matmul(out=pagg[:], lhsT=ET[:, sl], rhs=vnat[:, sl],
                                     start=(b == 0), stop=(b == 7))
                # out_h = h_own + agg / Z
                nc.vector.tensor_scalar(
                    out=oh[:], in0=pagg[:], scalar1=rZ[:], scalar2=None,
                    op0=mybir.AluOpType.mult,
                )
                nc.vector.tensor_tensor(out=oh[:], in0=oh[:], in1=ho[:],
                                        op=mybir.AluOpType.add)
                nc.sync.dma_start(out=out_h_d[:], in_=oh[:])

            # ---- phase C: gate rows via relu tiles + diag-strip matmuls ----
            # four [32, N] psum tiles (partition base 0), one per row-group
            with tc.tile_pool(name="ppgate", bufs=4, space="PSUM") as ppgate:
                w2cd_v = w2cd[:].rearrange("h (m c) -> h m c", m=32)
                pgs = [ppgate.tile([32, N], F32, tag="gate", name=f"pg{g}")
                       for g in range(4)]
                for i in range(R):
                    rt = rpool.tile([H, N], F32R, tag="rt")
                    if i in ACT_ROWS:
                        nc.scalar.activation(
                            rt[:], uT[:], mybir.ActivationFunctionType.Relu,
                            bias=uTbo[:, i:i + 1], scale=-1.0,
                        )
                    else:
                        nc.vector.tensor_scalar(
                            out=rt[:], in0=nuT[:], scalar1=uTbo[:, i:i + 1],
                            scalar2=0.0,
                            op0=mybir.AluOpType.add, op1=mybir.AluOpType.max,
                        )
                    g, m = divmod(i, 32)
                    for f in range(2):
                        nc.tensor.matmul(
                            out=pgs[g][:, f * 512:(f + 1) * 512],
                            lhsT=w2cd_v[:, m, :],
                            rhs=rt[:, f * 512:(f + 1) * 512],
                            start=(m == 0), stop=(m == 31),
                            skip_group_check=True,
                        )

                # s = E * (gate + c0)
                for g in range(4):
                    nc.vector.tensor_scalar(
                        out=s_sb[g * 32:(g + 1) * 32, :], in0=pgs[g][:],
                        scalar1=c0c[0:32, :], scalar2=None,
                        op0=mybir.AluOpType.add,
                    )
                nc.vector.tensor_tensor(out=s_sb[:], in0=s_sb[:], in1=E[:],
                                        op=mybir.AluOpType.mult)

            # ---- phase D: delta_x ----
            with (
                tc.tile_pool(name="pptp2", bufs=2, space="PSUM") as pptp2,
                tc.tile_pool(name="ppdel", bufs=1, space="PSUM") as ppdel,
            ):
                sT = wpool.tile([128, N], F32, tag="sT")
                for b in range(8):
                    sl = slice(b * 128, (b + 1) * 128)
                    pt = pptp2.tile([128, 128], F32, tag="tp2")
                    nc.tensor.transpose(pt[:], s_sb[:, sl], ident[:])
                    nc.vector.tensor_copy(sT[:, sl], pt[:])

                pd = ppdel.tile([R, 3], F32, tag="del")
                xaug_v = xaug[:].rearrange("p (b c) -> p b c", b=8)
                for b in range(8):
                    sl = slice(b * 128, (b + 1) * 128)
                    nc.tensor.matmul(out=pd[:], lhsT=sT[:, sl],
                                     rhs=xaug_v[:, b, :],
                                     start=(b == 0), stop=(b == 7))
                d_sb = wpool.tile([R, 3], F32, tag="d")
                nc.vector.tensor_copy(d_sb[:], pd[:])

                # delta = (S*x_own - s@x) / Z ; out_x = x_own + delta
                t1 = wpool.tile([R, 2], F32, tag="t1")
                nc.vector.tensor_scalar(
                    out=t1[:], in0=xo[:], scalar1=d_sb[:, 2:3], scalar2=None,
                    op0=mybir.AluOpType.mult,
                )
                nc.vector.tensor_tensor(out=t1[:], in0=t1[:], in1=d_sb[:, 0:2],
                                        op=mybir.AluOpType.subtract)
                nc.vector.tensor_scalar(
                    out=t1[:], in0=t1[:], scalar1=rZ[:], scalar2=None,
                    op0=mybir.AluOpType.mult,
                )
                ox = wpool.tile([R, 2], F32, tag="ox")
                nc.vector.tensor_tensor(out=ox[:], in0=t1[:], in1=xo[:],
                                        op=mybir.AluOpType.add)
                nc.sync.dma_start(out=out_x_d[:], in_=ox[:])

    nc.finalize()
    return nc


def make_in_maps(h, x, Wq, bq, Wk, bk, Wv, bv, We1, be1, We2, be2, Wc, bc):
    """Host-side staging: layout transforms + weight folding only."""
    f = np.float32
    h = np.asarray(h, f)
    x = np.asarray(x, f)
    s = 1.0 / np.sqrt(np.float32(H))
    Wq_s = np.ascontiguousarray(np.asarray(Wq, f) * s)
    bq_s = (np.asarray(bq, f) * s).reshape(H, 1)
    w2c = (np.asarray(We2, f) @ np.asarray(Wc, f)).reshape(H)  # [H]
    c0 = float(np.asarray(be2, f) @ np.asarray(Wc, f).reshape(H) + np.asarray(bc, f)[0])

    W2CD = np.zeros((H, 32, 32), f)
    for m in range(32):
        W2CD[:, m, m] = w2c
    W2CD = np.ascontiguousarray(W2CD.reshape(H, 1024))

    hT = np.ascontiguousarray(h.T)
    xT = np.ascontiguousarray(x.T)
    xaug = np.empty((128, 8, 3), f)
    xr = x.reshape(8, 128, 2)  # [b, p, c]
    xaug[:, :, 0:2] = xr.transpose(1, 0, 2)
    xaug[:, :, 2] = 1.0
    xaug = np.ascontiguousarray(xaug.reshape(128, 24))

    common = {
        "hT": hT,
        "x_aug": xaug,
        "xT": xT,
        "Wq_s": Wq_s,
        "Wk": np.ascontiguousarray(np.asarray(Wk, f)),
        "Wv": np.ascontiguousarray(np.asarray(Wv, f)),
        "We1": np.ascontiguousarray(np.asarray(We1, f)),
        "W2CD": W2CD,
        "ident": np.eye(128, dtype=f),
        "bq_col": bq_s,
        "bk_col": np.asarray(bk, f).reshape(H, 1),
        "be1_col": np.asarray(be1, f).reshape(H, 1),
        "c0_col": np.full((128, 1), c0, f),
        "bv_row": np.asarray(bv, f).reshape(1, H),
        "ones_col": np.ones((1, 128), f),
    }
    in_maps = []
    for c in range(NCORES):
        rows = slice(c * R, (c + 1) * R)
        m = dict(common)
        m["h_own"] = np.ascontiguousarray(h[rows])
        m["hT_own"] = np.ascontiguousarray(h[rows].T)
        m["x_own"] = np.ascontiguousarray(x[rows])
        m["xT_own"] = np.ascontiguousarray(x[rows].T)
        in_maps.append(m)
    return in_maps


_NC_CACHE = {}
LAST_RESULT = None


def kernel(h, x, batch, Wq, bq, Wk, bk, Wv, bv, We1, be1, We2, be2, Wc, bc):
    global LAST_RESULT
    if "nc" not in _NC_CACHE:
        _NC_CACHE["nc"] = build_nc()
    nc = _NC_CACHE["nc"]
    in_maps = make_in_maps(h, x, Wq, bq, Wk, bk, Wv, bv, We1, be1, We2, be2, Wc, bc)
    res = run_bass_kernel_spmd(nc, in_maps, list(range(NCORES)))
    LAST_RESULT = res
    out_h = np.concatenate([res.results[c]["out_h"] for c in range(NCORES)], axis=0)
    out_x = np.concatenate([res.results[c]["out_x"] for c in range(NCORES)], axis=0)
    return out_h, out_x
